# revision 18
# baseline (speedup 1.0000x reference)
import os
import sys

sys.path.insert(0, "/opt/trn_rl_repo")

import numpy as np
import ml_dtypes
BF16 = np.dtype(ml_dtypes.bfloat16)

O, T = 100000, 400000
D = 128
NCORES = 8
OWN = O // NCORES          # 12500 nodes owned per core
NG = 98                    # node groups of 128 per core (98*128 = 12544 >= 12500)
OPAD = 98 * 1024           # padded node count for A/B tables (100352)


def _remap(v):
    # A/B tables are stored in chunk-blocked layout [98, 128p, 8j, 128] so the
    # premultiply phase can write one contiguous [128, 1024] slab per chunk.
    # Flat row index of global node v in that layout:
    c8 = v // 1024
    p = v % 128
    j = (v % 1024) // 128
    return c8 * 1024 + p * 8 + j


def _build_graph(M):
    from concourse import bass, mybir, bacc
    import concourse.tile as tile
    from concourse.masks import make_identity

    f32 = mybir.dt.float32
    i32 = mybir.dt.int32
    LP = NG * M * 128

    nc = bacc.Bacc("TRN2", target_bir_lowering=False, debug=False, num_devices=NCORES)

    # ---- external inputs ----
    objT = nc.dram_tensor("objT", [128, OPAD], f32, kind="ExternalInput")
    predT1 = nc.dram_tensor("predT1", [128, LP], mybir.dt.bfloat16, kind="ExternalInput")
    predT2 = nc.dram_tensor("predT2", [128, LP], mybir.dt.bfloat16, kind="ExternalInput")
    gsB1 = nc.dram_tensor("gsB1", [128, NG * M], i32, kind="ExternalInput")
    gsA2 = nc.dram_tensor("gsA2", [128, NG * M], i32, kind="ExternalInput")
    objT_own = nc.dram_tensor("objT_own", [128, NG * 128], f32, kind="ExternalInput")
    s01t1 = nc.dram_tensor("s01t1", [NG, 128, M * 128], mybir.dt.bfloat16, kind="ExternalInput")
    s01t2 = nc.dram_tensor("s01t2", [NG, 128, M * 128], mybir.dt.bfloat16, kind="ExternalInput")
    bf16 = mybir.dt.bfloat16
    st1 = nc.dram_tensor("st1", [NG, 128, M * 128], bf16, kind="ExternalInput")
    st2 = nc.dram_tensor("st2", [NG, 128, M * 128], bf16, kind="ExternalInput")
    w1a = nc.dram_tensor("w1a", [384, 128], f32, kind="ExternalInput")
    w1ar_bf = nc.dram_tensor("w1ar_bf", [128, 128], mybir.dt.bfloat16, kind="ExternalInput")
    b1a = nc.dram_tensor("b1a", [1, 128], f32, kind="ExternalInput")
    w1b = nc.dram_tensor("w1b", [128, 384], f32, kind="ExternalInput")
    b1b = nc.dram_tensor("b1b", [1, 384], f32, kind="ExternalInput")
    w2a = nc.dram_tensor("w2a", [128, 128], f32, kind="ExternalInput")
    b2a = nc.dram_tensor("b2a", [1, 128], f32, kind="ExternalInput")
    w2b = nc.dram_tensor("w2b", [128, 128], f32, kind="ExternalInput")
    b2b = nc.dram_tensor("b2b", [1, 128], f32, kind="ExternalInput")

    # ---- external outputs ----
    out_objT = nc.dram_tensor("out_objT", [128, NG * 128], f32, kind="ExternalOutput")
    out_p1 = nc.dram_tensor("out_p1", [LP, 128], f32, kind="ExternalOutput")

    # ---- internal DRAM: premultiplied tables A = obj@W1a_s + b1a, B = obj@W1a_o
    a_tab = nc.dram_tensor("a_tab", [OPAD, 128], f32)
    b_tab = nc.dram_tensor("b_tab", [OPAD, 128], f32)

    with tile.TileContext(nc) as tc:
        # ---------- resident constants ----------
        with tc.tile_pool(name="const", bufs=1) as cpool:
            ident = cpool.tile([128, 128], f32)
            make_identity(nc, ident[:])
            ones1 = cpool.tile([1, 128], f32)
            nc.vector.memset(ones1[:], 1.0)
            w1a_s_sb = cpool.tile([128, 128], f32)
            nc.sync.dma_start(out=w1a_s_sb[:], in_=w1a[0:128, :])
            w1a_r_sb = cpool.tile([128, 128], mybir.dt.bfloat16)
            nc.sync.dma_start(out=w1a_r_sb[:], in_=w1ar_bf[:, :])
            w1a_o_sb = cpool.tile([128, 128], f32)
            nc.sync.dma_start(out=w1a_o_sb[:], in_=w1a[256:384, :])
            w1b_sb = cpool.tile([128, 384], f32)
            nc.sync.dma_start(out=w1b_sb[:], in_=w1b[:, :])
            w2a_sb = cpool.tile([128, 128], f32)
            nc.sync.dma_start(out=w2a_sb[:], in_=w2a[:, :])
            w2b_sb = cpool.tile([128, 128], f32)
            nc.sync.dma_start(out=w2b_sb[:], in_=w2b[:, :])
            b1a_sb = cpool.tile([1, 128], f32)
            nc.sync.dma_start(out=b1a_sb[:], in_=b1a[:, :])
            b1b_sb = cpool.tile([1, 384], f32)
            nc.sync.dma_start(out=b1b_sb[:], in_=b1b[:, :])
            b2a_sb = cpool.tile([1, 128], f32)
            nc.sync.dma_start(out=b2a_sb[:], in_=b2a[:, :])
            b2b_sb = cpool.tile([1, 128], f32)
            nc.sync.dma_start(out=b2b_sb[:], in_=b2b[:, :])
            # gather index tables stay resident in SBUF
            gsB1_sb = cpool.tile([128, NG * M], i32)
            nc.sync.dma_start(out=gsB1_sb[:], in_=gsB1[:, :])
            gsA2_sb = cpool.tile([128, NG * M], i32)
            nc.sync.dma_start(out=gsA2_sb[:], in_=gsA2[:, :])


            # ---------- phase A: premultiply A/B tables ----------
            with (
                tc.tile_pool(name="pa_in", bufs=3) as pain,
                tc.tile_pool(name="pa_out", bufs=4) as paout,
                tc.tile_pool(name="pa_ps", bufs=2, space="PSUM") as paps,
            ):
                for c8 in range(OPAD // 1024):
                    oslab = pain.tile([128, 1024], f32, tag="oslab")
                    nc.sync.dma_start(
                        out=oslab[:], in_=objT[:, c8 * 1024 : (c8 + 1) * 1024]
                    )
                    aslab = paout.tile([128, 1024], f32, tag="aslab")
                    bslab = paout.tile([128, 1024], f32, tag="bslab")
                    for j in range(8):
                        otile = oslab[:, j * 128 : (j + 1) * 128]
                        aps = paps.tile([128, 128], f32, tag="aps")
                        nc.tensor.matmul(
                            aps[:], lhsT=otile, rhs=w1a_s_sb[:],
                            start=True, stop=False,
                        )
                        nc.tensor.matmul(
                            aps[:], lhsT=ones1[:], rhs=b1a_sb[:],
                            start=False, stop=True,
                        )
                        bps = paps.tile([128, 128], f32, tag="bps")
                        nc.tensor.matmul(
                            bps[:], lhsT=otile, rhs=w1a_o_sb[:],
                            start=True, stop=True,
                        )
                        if j % 2 == 0:
                            nc.vector.tensor_copy(aslab[:, j * 128 : (j + 1) * 128], aps[:])
                            nc.scalar.activation(bslab[:, j * 128 : (j + 1) * 128], bps[:], mybir.ActivationFunctionType.Copy)
                        else:
                            nc.scalar.activation(aslab[:, j * 128 : (j + 1) * 128], aps[:], mybir.ActivationFunctionType.Copy)
                            nc.vector.tensor_copy(bslab[:, j * 128 : (j + 1) * 128], bps[:])
                    nc.sync.dma_start(
                        out=a_tab[c8 * 1024 : (c8 + 1) * 1024, :].rearrange(
                            "(p j) d -> p j d", p=128, j=8
                        ),
                        in_=aslab[:],
                    )
                    nc.sync.dma_start(
                        out=b_tab[c8 * 1024 : (c8 + 1) * 1024, :].rearrange(
                            "(p j) d -> p j d", p=128, j=8
                        ),
                        in_=bslab[:],
                    )

            # ---------- phase B: message passing + net2 ----------
            with (
                tc.tile_pool(name="gath", bufs=6) as gpool,
                tc.tile_pool(name="pred", bufs=3) as prpool,
                tc.tile_pool(name="stp", bufs=3) as stpool,
                tc.tile_pool(name="msg", bufs=4) as mpool,
                tc.tile_pool(name="hts", bufs=4) as hpool,
                tc.tile_pool(name="n2s", bufs=4) as n2pool,
                tc.tile_pool(name="ps_h", bufs=2, space="PSUM") as psh,
                tc.tile_pool(name="ps_l2", bufs=2, space="PSUM") as psl2,
                tc.tile_pool(name="ps_pool", bufs=1, space="PSUM") as pspool,
                tc.tile_pool(name="ps_own", bufs=1, space="PSUM") as psown,
                tc.tile_pool(name="ps_n2", bufs=1, space="PSUM") as psn2,
            ):
                for g in range(NG):
                    pooled_ps = pspool.tile([128, 128], f32, tag="pooled")
                    oslc = gpool.tile([128, 128], f32, tag="oslc")
                    nc.sync.dma_start(
                        out=oslc[:], in_=objT_own[:, g * 128 : (g + 1) * 128]
                    )
                    for l in (1, 2):
                        predT = predT1 if l == 1 else predT2
                        st = st1 if l == 1 else st2
                        s01t = s01t1 if l == 1 else s01t2
                        # sorted side: one gather of this group's own 128 rows,
                        # expanded to message slots via one-hot matmul
                        rnd_tab = b_tab if l == 1 else a_tab
                        rnd_idx = gsB1_sb if l == 1 else gsA2_sb
                        ownp = psown.tile([128, 128], f32, tag="ownp")
                        if l == 1:
                            nc.tensor.matmul(
                                ownp[:], lhsT=oslc[:], rhs=w1a_s_sb[:],
                                start=True, stop=False,
                            )
                            nc.tensor.matmul(
                                ownp[:], lhsT=ones1[:], rhs=b1a_sb[:],
                                start=False, stop=True,
                            )
                        else:
                            nc.tensor.matmul(
                                ownp[:], lhsT=oslc[:], rhs=w1a_o_sb[:],
                                start=True, stop=True,
                            )
                        ogb = gpool.tile([128, 128], mybir.dt.bfloat16, tag="ogb")
                        nc.vector.tensor_copy(ogb[:], ownp[:])
                        s01 = stpool.tile([128, M * 128], mybir.dt.bfloat16, tag="s01")
                        nc.sync.dma_start(out=s01[:], in_=s01t[g, :, :])
                        bg = gpool.tile([128, M * 128], f32, tag="bg")
                        for j in range(M):
                            nc.gpsimd.indirect_dma_start(
                                out=bg[:, j * 128 : (j + 1) * 128],
                                out_offset=None,
                                in_=rnd_tab[:, :],
                                in_offset=bass.IndirectOffsetOnAxis(
                                    ap=rnd_idx[:, g * M + j : g * M + j + 1], axis=0
                                ),
                            )
                        prs = prpool.tile([128, M * 128], mybir.dt.bfloat16, tag="prs")
                        nc.sync.dma_start(
                            out=prs[:], in_=predT[:, g * M * 128 : (g + 1) * M * 128]
                        )
                        sts = stpool.tile([128, M * 128], mybir.dt.bfloat16, tag="sts")
                        nc.sync.dma_start(out=sts[:], in_=st[g, :, :])
                        if l == 1:
                            msl = mpool.tile([128, M * 128], mybir.dt.bfloat16, tag="ms")
                            npl = mpool.tile([128, M * 128], f32, tag="np")
                        else:
                            msl = mpool.tile([128, M * 128], mybir.dt.bfloat16, tag="mo")
                            npl = None
                        for j in range(M):
                            sl = slice(j * 128, (j + 1) * 128)
                            hps = psh.tile([128, 128], f32, tag="hps")
                            # H^T = Own^T(expand) + Rnd^T + W1a_r^T @ predT
                            nc.tensor.matmul(
                                hps[:], lhsT=ogb[:], rhs=s01[:, sl],
                                start=True, stop=False, skip_group_check=True,
                            )
                            nc.tensor.matmul(
                                hps[:], lhsT=bg[:, sl], rhs=ident[:],
                                is_transpose=True, start=False, stop=False,
                                skip_group_check=True,
                            )
                            nc.tensor.matmul(
                                hps[:], lhsT=w1a_r_sb[:], rhs=prs[:, sl],
                                start=False, stop=True, skip_group_check=True,
                            )
                            hsb = hpool.tile([128, 128], f32, tag="hsb")
                            nc.scalar.activation(
                                hsb[:], hps[:], mybir.ActivationFunctionType.Relu
                            )
                            if l == 1:
                                l2 = psl2.tile([128, 256], f32, tag="l2a")
                                nc.tensor.matmul(
                                    l2[:], lhsT=hsb[:], rhs=w1b_sb[:, 0:256],
                                    start=True, stop=False,
                                )
                                nc.tensor.matmul(
                                    l2[:], lhsT=ones1[:], rhs=b1b_sb[:, 0:256],
                                    start=False, stop=True,
                                )
                                nc.vector.tensor_copy(msl[:, sl], l2[:, 0:128])
                                nc.vector.tensor_copy(npl[:, sl], l2[:, 128:256])
                            else:
                                l2 = psl2.tile([128, 256], f32, tag="l2a")
                                nc.tensor.matmul(
                                    l2[:, 0:128], lhsT=hsb[:], rhs=w1b_sb[:, 256:384],
                                    start=True, stop=False,
                                )
                                nc.tensor.matmul(
                                    l2[:, 0:128], lhsT=ones1[:], rhs=b1b_sb[:, 256:384],
                                    start=False, stop=True,
                                )
                                nc.vector.tensor_copy(msl[:, sl], l2[:, 0:128])
                            # pooled^T[c, n] += sum_m msg[m, c] * St[m, n]
                            nc.tensor.matmul(
                                pooled_ps[:], lhsT=msl[:, sl], rhs=sts[:, sl],
                                start=(l == 1 and j == 0),
                                stop=(l == 2 and j == M - 1),
                                skip_group_check=True,
                            )
                        if l == 1:
                            nc.sync.dma_start(
                                out=out_p1[g * M * 128 : (g + 1) * M * 128, :].rearrange(
                                    "(j p) d -> p j d", p=128, j=M
                                ),
                                in_=npl[:],
                            )
                    # ---- net2 on this group's pooled^T ----
                    pooled_sb = n2pool.tile([128, 128], f32, tag="pool_sb")
                    nc.vector.tensor_copy(pooled_sb[:], pooled_ps[:])
                    ups = psn2.tile([128, 128], f32, tag="ups")
                    nc.tensor.matmul(
                        ups[:], lhsT=w2a_sb[:], rhs=pooled_sb[:], start=True, stop=False
                    )
                    nc.tensor.matmul(
                        ups[:], lhsT=b2a_sb[:], rhs=ones1[:], start=False, stop=True
                    )
                    usb = n2pool.tile([128, 128], f32, tag="usb")
                    nc.scalar.activation(
                        usb[:], ups[:], mybir.ActivationFunctionType.Relu
                    )
                    ops = psn2.tile([128, 128], f32, tag="ops")
                    nc.tensor.matmul(
                        ops[:], lhsT=w2b_sb[:], rhs=usb[:], start=True, stop=False
                    )
                    nc.tensor.matmul(
                        ops[:], lhsT=b2b_sb[:], rhs=ones1[:], start=False, stop=True
                    )
                    osb = n2pool.tile([128, 128], f32, tag="osb")
                    nc.vector.tensor_copy(osb[:], ops[:])
                    nc.sync.dma_start(
                        out=out_objT[:, g * 128 : (g + 1) * 128], in_=osb[:]
                    )
    nc.compile()
    return nc


def kernel(obj_vecs, pred_vecs, edges, W1a, b1a, W1b, b1b, W2a, b2a, W2b, b2b):
    from concourse.bass_utils import run_bass_kernel_spmd

    obj_vecs = np.asarray(obj_vecs, np.float32)
    pred_vecs = np.asarray(pred_vecs, np.float32)
    edges = np.asarray(edges)
    s_idx = edges[:, 0].astype(np.int64)
    o_idx = edges[:, 1].astype(np.int64)

    counts = np.bincount(s_idx, minlength=O) + np.bincount(o_idx, minlength=O)
    recip = (1.0 / np.maximum(counts, 1.0)).astype(np.float32)

    # per (core, group, list) message slot assignment
    key = {1: s_idx, 2: o_idx}
    per_core = []
    maxcnt = 0
    for c in range(NCORES):
        lists = {}
        for l in (1, 2):
            k = key[l]
            sel = np.where((k >= c * OWN) & (k < (c + 1) * OWN))[0]
            loc = k[sel] - c * OWN
            order = np.argsort(loc, kind="stable")
            eids = sel[order]
            locs = loc[order]
            grp = locs >> 7
            cnts = np.bincount(grp, minlength=NG)
            maxcnt = max(maxcnt, int(cnts.max()))
            lists[l] = (eids, locs, grp, cnts)
        per_core.append(lists)
    M = max(1, -(-maxcnt // 128))
    LP = NG * M * 128

    objT = np.zeros((128, OPAD), np.float32)
    objT[:, :O] = obj_vecs.T

    # remap of global node id into the blocked A/B table layout
    vv = np.arange(O, dtype=np.int64)
    remap_all = ((vv // 1024) * 1024 + (vv % 128) * 8 + (vv % 1024) // 128).astype(
        np.int32
    )

    in_maps = []
    host_meta = []
    for c in range(NCORES):
        im = {
            "objT": objT,
            "w1a": np.asarray(W1a, np.float32),
            "w1ar_bf": np.asarray(W1a, np.float32)[128:256].astype(BF16),
            "b1a": np.asarray(b1a, np.float32).reshape(1, 128),
            "w1b": np.asarray(W1b, np.float32),
            "b1b": np.asarray(b1b, np.float32).reshape(1, 384),
            "w2a": np.asarray(W2a, np.float32),
            "b2a": np.asarray(b2a, np.float32).reshape(1, 128),
            "w2b": np.asarray(W2b, np.float32),
            "b2b": np.asarray(b2b, np.float32).reshape(1, 128),
        }
        meta = {}
        for l in (1, 2):
            eids, locs, grp, cnts = per_core[c][l]
            # slot index within each group
            starts = np.zeros(NG, np.int64)
            starts[1:] = np.cumsum(cnts)[:-1]
            within = np.arange(len(eids)) - starts[grp]
            slot = grp * (M * 128) + within  # slot in [0, LP)
            j = (slot % (M * 128)) // 128
            p = slot % 128
            gcol = (slot // (M * 128)) * M + j

            eid_full = np.full(LP, -1, np.int64)
            eid_full[slot] = eids

            gs_rnd = np.zeros((128, NG * M), np.int32)
            rnd_key = o_idx if l == 1 else s_idx
            gs_rnd[p, gcol] = remap_all[rnd_key[eids]]
            s01 = np.zeros((NG, 128, M * 128), BF16)
            s01[grp, locs & 127, j * 128 + p] = np.ones(len(eids), BF16)

            predT = np.zeros((128, LP), BF16)
            pr = np.zeros((LP, 128), np.float32)
            pr[slot] = pred_vecs[eids]
            predT[:, :] = pr.T.astype(BF16)

            st = np.zeros((NG, 128, M * 128), BF16)
            st[grp, p, j * 128 + (locs & 127)] = recip[key[l][eids]].astype(BF16)

            sfx = str(l)
            im["gsB1" if l == 1 else "gsA2"] = gs_rnd
            im["s01t" + sfx] = s01
            im["predT" + sfx] = predT
            im["st" + sfx] = st
            meta[l] = eid_full
        im["objT_own"] = np.ascontiguousarray(
            objT[:, c * OWN : c * OWN + NG * 128]
        )
        in_maps.append(im)
        host_meta.append(meta)

    print(f"[kernel] host prep done, M={M}", flush=True)
    nc = _build_graph(M)
    print("[kernel] graph built+compiled", flush=True)
    res = run_bass_kernel_spmd(
        nc, in_maps, core_ids=list(range(NCORES)), trace=bool(os.environ.get("K_TRACE"))
    )
    kernel.last_results = res
    kernel.last_in_maps = in_maps
    kernel.last_meta = host_meta
    kernel.last_M = M
    print("[kernel] hw run done", flush=True)

    new_obj = np.empty((O, D), np.float32)
    new_p = np.empty((T, D), np.float32)
    for c in range(NCORES):
        r = res.results[c]
        new_obj[c * OWN : (c + 1) * OWN] = r["out_objT"].T[:OWN]
        eid_full = host_meta[c][1]
        mask = eid_full >= 0
        new_p[eid_full[mask]] = r["out_p1"][mask]
    return new_obj, new_p
